# revision 42
# baseline (speedup 1.0000x reference)
"""Trainium2 Bass kernel for a 2-layer GCN encoder (AssemblyQueryEncoder).

Reference computation (PyG-style GCNConv x2 + global mean pool + linear + L2norm):
    h1 = relu(gcnconv(x, W1, b1));  h2 = relu(gcnconv(h1, W2, b2))
    g  = segment_mean(h2, batch) @ Wl + bl;  out = g / max(||g||_2, eps)

Distribution over 8 NeuronCores:
  - Nodes sharded contiguously (5120 padded/core); each core owns the incoming
    edges of its nodes (destination partitioning).
  - Norm folding: dinv[src] is folded into the gather-table rows (pre-scaled
    x / scaled transpose for layer 2), dinv[dst] is applied as a per-partition
    activation scale on the aggregation PSUM.  Per-edge selection matrices are
    therefore 0/1 one-hot and generated ON-CHIP (DVE is_equal against an iota
    row) from a 2-byte dstcol stream; nothing dense is streamed from DRAM.
  - Self-loops ride as ordinary gathered self-edges.
  - Aggregation is linear, so the weight transform runs AFTER aggregation:
    the layer-1 table is just dinv*x — a pure host-built parameter (zero
    kernel time before gathers start) — and layer 2 gathers raw dinv*h1.
    Each block's epilogue does scaled-transpose (diag-dinv matmul, folding
    dinv[dst]) -> @W -> rank-1 bias matmul -> Relu; psum->bf16 staging
    copies run on the scalar engine so the DVE keeps the one-hot stream.
    Layer 2's table AllGather is split in 2 halves fired mid-layer-1 and its
    aggregation is two-pass (stream 0 vs 1) to hide collective latency.
  - Tables are split in 2 halves (<=20480 rows) so dma_gather int16 indices
    cover them; gathers are issued in 8-tile (1024-index) calls — the SWDGE
    descriptor ring holds exactly 1024 descriptors, larger calls deadlock —
    round-robined over the 4 SWDGE queues with an 8-deep buffer pipeline.
    Pad slots gather spread-out throwaway rows: same-address pad gathers
    serialize the DMA drain and were the dominant cost at one point.
  - Pooled per-graph sums (1/count folded into the pooling matrix) are
    AllReduced ([128,64]); final linear + L2 norm computed redundantly in f32.
"""

import sys

sys.path.insert(0, "/opt/trn_rl_repo")

import numpy as np

P = 128  # partitions


def _cdiv(a, b):
    return (a + b - 1) // b


class GCNConfig:
    def __init__(self, n_nodes=40000, n_graphs=64, d_in=128, d_hid=128, d_out=64,
                 n_cores=8, gch=8, sgen=8):
        self.n_nodes = n_nodes
        self.n_graphs = n_graphs
        self.d_in = d_in
        self.d_hid = d_hid
        self.d_out = d_out
        self.n_cores = n_cores
        self.gch = gch      # gather chunk (tiles per dma_gather call)
        self.sgen = sgen    # one-hot generation chunk (tiles per DVE op)
        self.nloc = _cdiv(n_nodes, n_cores * P) * P  # padded nodes per core
        self.npad = self.nloc * n_cores
        self.nblk = self.nloc // P   # 128-node blocks per core (40)
        self.nh = 2                  # table halves
        self.hs = self.nloc // self.nh           # rows per half per core (2560)
        self.hrows = self.hs * n_cores           # rows per half table (20480)
        assert self.hrows <= 32768  # int16 gather indices
        assert self.hs % P == 0


def _wrap_idx(flat):
    """dma_gather index layout: element i -> [i % 16, i // 16], x8 partitions."""
    n = flat.shape[0]
    assert n % 16 == 0
    arr = np.zeros((16, n // 16), np.int16)
    arr[np.arange(n) % 16, np.arange(n) // 16] = flat
    return np.tile(arr, (8, 1))


def preprocess(cfg, x, edge_index, batch):
    """Host-side preprocessing.  Edges (plus one self-edge per real node) are
    grouped per core by destination block and split into nh streams by source
    half; each (block, stream) list is padded to a tile multiple shared by all
    cores.  Streams carry int16 gather rows + bf16 destination columns."""
    import ml_dtypes
    bfd = ml_dtypes.bfloat16

    n, nc_ = cfg.n_nodes, cfg.n_cores
    nh, hs = cfg.nh, cfg.hs
    src_a = np.asarray(edge_index[0], dtype=np.int64)
    dst_a = np.asarray(edge_index[1], dtype=np.int64)
    batch = np.asarray(batch, dtype=np.int64)

    deg = np.bincount(dst_a, minlength=n).astype(np.float64) + 1.0
    dinv = 1.0 / np.sqrt(deg)

    # append self-edges (i -> i); all folded coefficients are 1 in the
    # one-hot scheme (dinv[src] in the table, dinv[dst] in the epilogue)
    selfs = np.arange(n, dtype=np.int64)
    src_a = np.concatenate([src_a, selfs])
    dst_a = np.concatenate([dst_a, selfs])

    # source half + row within the half table (rank-major concat layout)
    h_a = (src_a % cfg.nloc) // hs
    row_a = (src_a // cfg.nloc) * hs + (src_a % hs)

    order = np.lexsort((dst_a, h_a))
    src_h = h_a[order]
    dst_s = dst_a[order]
    row_s = row_a[order]
    hstart = np.searchsorted(src_h, np.arange(nh + 1))

    nblk_g = cfg.npad // P
    res = {"T": [], "ttot": []}
    for h in range(nh):
        lo_, hi_ = hstart[h], hstart[h + 1]
        s_r = row_s[lo_:hi_]
        s_d = dst_s[lo_:hi_]
        blk = s_d // P
        counts = np.bincount(blk, minlength=nblk_g).reshape(nc_, cfg.nblk)
        T = _cdiv(counts.max(axis=0), P).astype(np.int64)
        ttot = max(int(T.sum()), 1)
        tstart = np.concatenate([[0], np.cumsum(T)]).astype(np.int64)
        bstart = np.concatenate(
            [[0], np.cumsum(np.bincount(blk, minlength=nblk_g))]).astype(np.int64)
        # Pad slots gather a throwaway row (their one-hot column is zeroed);
        # spread them over distinct rows — same-address gathers serialize on
        # one DRAM bank and made the pad-heavy core ~25% slower.
        spread = (np.arange(P)[:, None] * 577 + np.arange(ttot)[None, :] * 131
                  ) % (hs * nc_)
        gidx = np.broadcast_to(spread.astype(np.int16),
                               (nc_, P, ttot)).copy()
        dcol = np.full((nc_, P, ttot), -1.0, bfd)
        for c in range(nc_):
            for b in range(cfg.nblk):
                gb = c * cfg.nblk + b
                e0, e1 = bstart[gb], bstart[gb + 1]
                m = e1 - e0
                if m == 0:
                    continue
                jj = np.arange(m)
                pp, tt = jj % P, tstart[b] + jj // P
                gidx[c, pp, tt] = s_r[e0:e1]
                dcol[c, pp, tt] = (s_d[e0:e1] % P).astype(bfd)
        widx = np.stack([_wrap_idx(gidx[c].T.reshape(-1)) for c in range(nc_)])
        res[f"gidx{h}"] = widx
        res[f"dcol{h}"] = dcol
        res["T"].append(T)
        res["ttot"].append(ttot)

    # per-core constants
    d_all = np.zeros(cfg.npad, np.float64)
    d_all[:n] = dinv
    # dg: per-block diagonal dinv (own nodes) for the scaled transpose
    dg = np.zeros((nc_, P, cfg.nblk * P), bfd)
    # dinvc: [P, nblk] f32 post-aggregation scale (own nodes)
    dinvc = np.zeros((nc_, P, cfg.nblk), np.float32)
    # invd: [1, nloc] bf16 sqrt(deg) for the pre-scaled bias (own nodes)
    invd = np.zeros((nc_, 1, cfg.nloc), bfd)
    for c in range(nc_):
        loc = d_all[c * cfg.nloc:(c + 1) * cfg.nloc]
        for b in range(cfg.nblk):
            dg[c, np.arange(P), b * P + np.arange(P)] = \
                loc[b * P:(b + 1) * P].astype(bfd)
            dinvc[c, :, b] = loc[b * P:(b + 1) * P].astype(np.float32)
        nz = loc > 0
        invd[c, 0, nz] = (1.0 / loc[nz]).astype(bfd)

    # layer-1 gather table: dinv-scaled x rows (aggregation is linear, so
    # the W1 transform moves AFTER aggregation and the table is a pure
    # host-built parameter), rank-major half layout, replicated
    xf = np.asarray(x, dtype=np.float32)
    xsc = np.zeros((cfg.npad, cfg.d_in), bfd)
    xsc[:n] = (xf * dinv[:, None].astype(np.float32)).astype(bfd)
    t1 = [np.zeros((cfg.hrows, cfg.d_in), bfd) for _ in range(nh)]
    for r in range(nc_):
        for h in range(nh):
            t1[h][r * hs:(r + 1) * hs] = \
                xsc[r * cfg.nloc + h * hs:r * cfg.nloc + (h + 1) * hs]

    # pooling matrix with 1/count folded in, block-major [P, nblk*G], bf16
    g_ = cfg.n_graphs
    cnt = np.maximum(np.bincount(batch, minlength=g_).astype(np.float32), 1.0)
    pm = np.zeros((nc_, P, cfg.nblk * g_), bfd)
    for c in range(nc_):
        for b in range(cfg.nblk):
            base = c * cfg.nloc + b * P
            hi2 = min(base + P, n)
            if hi2 <= base:
                continue
            rows = np.arange(hi2 - base)
            gg = batch[base:hi2]
            pm[c, rows, b * g_ + gg] = (1.0 / cnt[gg]).astype(bfd)

    res.update(t1h0=t1[0], t1h1=t1[1], pm=pm, dg=dg, dinvc=dinvc)
    return res


def build(cfg, Ts, ttots):
    """Build the SPMD Bass graph (same program for all cores)."""
    import concourse.mybir as mybir
    import concourse.tile as tile
    from concourse import bacc, bass

    f32 = mybir.dt.float32
    bf = mybir.dt.bfloat16
    i16 = mybir.dt.int16
    AF = mybir.ActivationFunctionType
    ALU = mybir.AluOpType

    nc_ = cfg.n_cores
    nblk = cfg.nblk
    nloc = cfg.nloc
    nh, hs, hrows = cfg.nh, cfg.hs, cfg.hrows
    dh = cfg.d_hid
    do = cfg.d_out
    g_ = cfg.n_graphs
    rg = [list(range(nc_))]
    GCH, SGEN = cfg.gch, cfg.sgen
    tstarts = [np.concatenate([[0], np.cumsum(T)]).astype(np.int64) for T in Ts]

    nc = bacc.Bacc("TRN2", target_bir_lowering=False, debug=False,
                   num_devices=nc_, num_swdge_queues=4)

    # ---- parameters ----
    t1p = [nc.declare_dram_parameter(f"t1h{h}", [hrows, dh], bf,
                                     isOutput=False) for h in range(nh)]
    gidx_p, dcol_p = [], []
    for h in range(nh):
        gidx_p.append(nc.declare_dram_parameter(
            f"gidx{h}", [P, ttots[h] * 8], i16, isOutput=False))
        dcol_p.append(nc.declare_dram_parameter(
            f"dcol{h}", [P, ttots[h]], bf, isOutput=False))
    dg_p = nc.declare_dram_parameter("dg", [P, nblk * P], bf, isOutput=False)
    dinvc_p = nc.declare_dram_parameter("dinvc", [P, nblk], f32, isOutput=False)
    pm_p = nc.declare_dram_parameter("pm", [P, nblk * g_], bf, isOutput=False)
    w1_p = nc.declare_dram_parameter("W1", [cfg.d_in, dh], bf, isOutput=False)
    w2_p = nc.declare_dram_parameter("W2", [dh, dh], bf, isOutput=False)
    wl_p = nc.declare_dram_parameter("Wl", [dh, do], f32, isOutput=False)
    b1_p = nc.declare_dram_parameter("b1", [1, dh], bf, isOutput=False)
    b2_p = nc.declare_dram_parameter("b2", [1, dh], bf, isOutput=False)
    bl_p = nc.declare_dram_parameter("bl", [1, do], f32, isOutput=False)
    out_p = nc.declare_dram_parameter("out", [g_, do], f32, isOutput=True)

    # ---- internal DRAM ----
    ag2h = [nc.dram_tensor(f"ag2h{h}", [hs, dh], bf) for h in range(nh)]
    t2h = [nc.dram_tensor(f"t2h{h}", [hrows, dh], bf, addr_space="Shared")
           for h in range(nh)]
    arin = [nc.dram_tensor(f"arin{i}", [dh, g_], f32) for i in (0, 1)]
    arout = [nc.dram_tensor(f"arout{i}", [dh, g_], f32, addr_space="Shared")
             for i in (0, 1)]

    with tile.TileContext(nc) as tc:
        with (
            tc.tile_pool(name="const", bufs=1) as cpool,
            tc.tile_pool(name="big", bufs=1) as bigpool,
            tc.tile_pool(name="gat", bufs=8) as gpool,
            tc.tile_pool(name="sel", bufs=4) as selpool,
            tc.tile_pool(name="blk", bufs=3) as blkpool,
            tc.tile_pool(name="small", bufs=2) as spool,
            tc.tile_pool(name="psum_a", bufs=3, space="PSUM") as pspool,
            tc.tile_pool(name="psum_e", bufs=2, space="PSUM") as pspool_e,
            tc.tile_pool(name="psum1", bufs=1, space="PSUM") as pspool1,
        ):
            # ---- constants ----
            w1_sb = cpool.tile([cfg.d_in, dh], bf)
            w2_sb = cpool.tile([dh, dh], bf)
            wl_sb = cpool.tile([dh, do], f32)
            b1_sb = cpool.tile([1, dh], bf)
            b2_sb = cpool.tile([1, dh], bf)
            bl_sb = cpool.tile([1, do], f32)
            onesf_sb = cpool.tile([1, P], f32)
            ones_b = cpool.tile([1, P], bf)
            iota_i = cpool.tile([P, P], i16)
            iota_bf = cpool.tile([P, P], bf)
            nc.sync.dma_start(w1_sb[:], w1_p[:])
            nc.sync.dma_start(w2_sb[:], w2_p[:])
            nc.sync.dma_start(wl_sb[:], wl_p[:])
            nc.sync.dma_start(b1_sb[:], b1_p[:])
            nc.sync.dma_start(b2_sb[:], b2_p[:])
            nc.sync.dma_start(bl_sb[:], bl_p[:])
            nc.gpsimd.memset(onesf_sb[:], 1.0)
            nc.gpsimd.memset(ones_b[:], 1.0)
            nc.gpsimd.iota(iota_i[:], pattern=[[1, P]], base=0,
                           channel_multiplier=0)
            nc.gpsimd.tensor_copy(iota_bf[:], iota_i[:])

            gidx_sb, dcol_sb = [], []
            for h in range(nh):
                t = bigpool.tile([P, ttots[h] * 8], i16, tag=f"gidx{h}")
                gidx_sb.append(t)
                t = bigpool.tile([P, ttots[h]], bf, tag=f"dcol{h}")
                dcol_sb.append(t)
            dg_sb = bigpool.tile([P, nblk * P], bf)
            dinvc_sb = bigpool.tile([P, nblk], f32)
            pm_sb = bigpool.tile([P, nblk * g_], bf)
            h2acc = bigpool.tile([P, nloc], f32)

            def bsl(b, w=P):
                return slice(b * w, (b + 1) * w)

            for h in range(nh):
                w4 = (ttots[h] * 8 + 3) // 4
                for q4 in range(4):
                    lo4 = q4 * w4
                    hi4 = min((q4 + 1) * w4, ttots[h] * 8)
                    if hi4 > lo4:
                        nc.scalar.dma_start(gidx_sb[h][:, lo4:hi4],
                                            gidx_p[h][:, lo4:hi4])
                nc.scalar.dma_start(dcol_sb[h][:], dcol_p[h][:])
            nc.scalar.dma_start(dg_sb[:], dg_p[:])
            nc.scalar.dma_start(dinvc_sb[:], dinvc_p[:])
            nc.scalar.dma_start(pm_sb[:], pm_p[:])

            # ---- streamed gather + on-chip one-hot machinery ----
            def make_streams(tables):
                sts = []
                for h in range(nh):
                    sts.append(dict(
                        tstart=tstarts[h], ttot=ttots[h], gidx=gidx_sb[h],
                        dcol=dcol_sb[h], view=tables[h][:],
                        gcur=None, gc0=-1, scur=None, sc0=-1, h=h))
                return sts

            qcnt = [0]

            def fetch(st, t):
                # gather chunk
                c0 = (t // GCH) * GCH
                if st["gc0"] != c0:
                    k = min(GCH, st["ttot"] - c0)
                    gt = gpool.tile([P, GCH, dh], bf, tag="g")
                    nc.gpsimd.dma_gather(
                        out_ap=gt[:, :k, :],
                        in_ap=st["view"],
                        idxs_ap=st["gidx"][:, c0 * 8:(c0 + k) * 8],
                        num_idxs=k * P,
                        num_idxs_reg=k * P,
                        elem_size=dh,
                        queue_num=qcnt[0] % 4,
                    )
                    qcnt[0] += 1
                    st["gcur"], st["gc0"] = gt, c0
                # one-hot selection chunk
                s0 = (t // SGEN) * SGEN
                if st["sc0"] != s0:
                    k2 = min(SGEN, st["ttot"] - s0)
                    sl = selpool.tile([P, SGEN * P], bf, tag="s")
                    dc = st["dcol"][:, s0:s0 + k2]
                    in0 = dc.to_broadcast([P, k2, P])
                    ib = iota_bf[:]
                    in1 = bass.AP(ib.tensor, ib.offset,
                                  [ib.ap[0], [0, k2], ib.ap[1]])
                    out = sl[:, :k2 * P].rearrange("p (a b) -> p a b", b=P)
                    nc.vector.tensor_tensor(out=out, in0=in0, in1=in1,
                                            op=ALU.is_equal)
                    st["scur"], st["sc0"] = sl, s0
                return (st["scur"][:, bsl(t - st["sc0"])],
                        st["gcur"][:, t - st["gc0"], :])

            def pass_a(st0):
                # stream-0-only accumulation of every block into h2acc, so it
                # can run while the other half-table is still being produced
                ts0 = st0["tstart"]
                for b in range(nblk):
                    if ts0[b + 1] == ts0[b]:
                        nc.vector.memset(h2acc[:, bsl(b)], 0.0)
                        continue
                    ps = pspool.tile([P, dh], f32, tag="psa")
                    for t in range(int(ts0[b]), int(ts0[b + 1])):
                        m_ap, g_ap = fetch(st0, t)
                        nc.tensor.matmul(ps[:], m_ap, g_ap,
                                         start=(t == ts0[b]),
                                         stop=(t == ts0[b + 1] - 1))
                    nc.vector.tensor_copy(h2acc[:, bsl(b)], ps[:])

            # ---- layer 1: aggregate raw dinv*x rows, then per-block
            # scaled-transpose (folds dinv[dst]) -> @W1 + bias -> relu; the
            # layer-2 table row dinv*h1 ships via a fused relu-with-scale ----
            streams1 = make_streams(t1p)
            for b in range(nblk):
                ps = pspool.tile([P, dh], f32, tag="psa")
                nt = int(tstarts[0][b + 1] - tstarts[0][b]
                         + tstarts[1][b + 1] - tstarts[1][b])
                k = 0
                for st in streams1:
                    ts = st["tstart"]
                    for t in range(int(ts[b]), int(ts[b + 1])):
                        m_ap, g_ap = fetch(st, t)
                        nc.tensor.matmul(ps[:], m_ap, g_ap,
                                         start=(k == 0), stop=(k == nt - 1))
                        k += 1
                pre_sb = blkpool.tile([P, dh], bf, tag="pre")
                nc.scalar.activation(pre_sb[:], ps[:], AF.Copy)
                psT = pspool_e.tile([P, dh], f32, tag="pse")
                nc.tensor.matmul(psT[:], pre_sb[:], dg_sb[:, bsl(b)],
                                 start=True, stop=True)
                preT = blkpool.tile([P, dh], bf, tag="preT")
                nc.scalar.activation(preT[:], psT[:], AF.Copy)
                ps2 = pspool_e.tile([P, dh], f32, tag="pse")
                nc.tensor.matmul(ps2[:], preT[:], w1_sb[:],
                                 start=True, stop=False)
                nc.tensor.matmul(ps2[:], ones_b[:], b1_sb[:],
                                 start=False, stop=True)
                a2s = blkpool.tile([P, dh], bf, tag="a2s")
                nc.scalar.activation(a2s[:], ps2[:], AF.Relu,
                                     scale=dinvc_sb[:, b:b + 1])
                hh, bb = b // (nblk // nh), b % (nblk // nh)
                nc.sync.dma_start(
                    ag2h[hh][bb * P:(bb + 1) * P, :], a2s[:])
                if b == nblk // nh - 1 or b == nblk - 1:
                    nc.gpsimd.collective_compute(
                        "AllGather", mybir.AluOpType.bypass,
                        replica_groups=rg, ins=[ag2h[hh][:]],
                        outs=[t2h[hh][:]])

            # ---- layer 2 aggregation: two passes ----
            streams2 = make_streams(t2h)
            st0, st1 = streams2
            ts1 = st1["tstart"]
            pass_a(st0)
            psp = pspool1.tile([P, g_], f32, tag="pool")
            for b in range(nblk):  # pass B: stream-1 accumulate + transform
                nt1 = int(ts1[b + 1] - ts1[b])
                if nt1 > 0:
                    ps = pspool.tile([P, dh], f32, tag="psa")
                    for t in range(int(ts1[b]), int(ts1[b + 1])):
                        m_ap, g_ap = fetch(st1, t)
                        nc.tensor.matmul(ps[:], m_ap, g_ap,
                                         start=(t == ts1[b]),
                                         stop=(t == ts1[b + 1] - 1))
                    nc.vector.tensor_tensor(out=h2acc[:, bsl(b)],
                                            in0=h2acc[:, bsl(b)], in1=ps[:],
                                            op=ALU.add)
                pre2 = blkpool.tile([P, dh], bf, tag="pre")
                nc.scalar.activation(pre2[:], h2acc[:, bsl(b)], AF.Copy)
                psT = pspool_e.tile([P, dh], f32, tag="pse")
                nc.tensor.matmul(psT[:], pre2[:], dg_sb[:, bsl(b)],
                                 start=True, stop=True)
                pre2T = blkpool.tile([P, dh], bf, tag="preT")
                nc.scalar.activation(pre2T[:], psT[:], AF.Copy)
                ps3 = pspool_e.tile([P, dh], f32, tag="pse")
                nc.tensor.matmul(ps3[:], pre2T[:], w2_sb[:],
                                 start=True, stop=False)
                nc.tensor.matmul(ps3[:], ones_b[:], b2_sb[:],
                                 start=False, stop=True)
                h2b = blkpool.tile([P, dh], bf, tag="h2")
                nc.scalar.activation(h2b[:], ps3[:], AF.Relu)
                nc.tensor.matmul(psp[:], h2b[:], pm_sb[:, bsl(b, g_)],
                                 start=(b % (nblk // 2) == 0),
                                 stop=(b % (nblk // 2) == nblk // 2 - 1))
                if b == nblk // 2 - 1 or b == nblk - 1:
                    # AllReduce each half of the pooled sums as soon as it is
                    # ready; the first one hides under the remaining blocks
                    half = 0 if b < nblk // 2 else 1
                    pool_sb = spool.tile([dh, g_], f32, tag=f"pl{half}")
                    nc.vector.tensor_copy(pool_sb[:], psp[:])
                    nc.gpsimd.dma_start(arin[half][:], pool_sb[:])
                    nc.gpsimd.collective_compute(
                        "AllReduce", mybir.AluOpType.add, replica_groups=rg,
                        ins=[arin[half][:]], outs=[arout[half][:]])
                    if half == 0:
                        psp = pspool1.tile([P, g_], f32, tag="pool")

            # ---- final linear: partial product per AllReduce half ----
            psg = pspool1.tile([g_, do], f32, tag="fin")
            m0 = spool.tile([dh, g_], f32, tag="m0")
            nc.sync.dma_start(m0[:], arout[0][:])
            nc.tensor.matmul(psg[:], m0[:], wl_sb[:], start=True,
                             stop=False)
            m1 = spool.tile([dh, g_], f32, tag="m1")
            nc.sync.dma_start(m1[:], arout[1][:])
            nc.tensor.matmul(psg[:], m1[:], wl_sb[:], start=False,
                             stop=False)
            nc.tensor.matmul(psg[:], onesf_sb[:, :g_], bl_sb[:],
                             start=False, stop=True)
            g_sb = spool.tile([g_, do], f32)
            nc.vector.tensor_copy(g_sb[:], psg[:])

            # ---- L2 normalize rows ----
            sq_sb = spool.tile([g_, do], f32)
            s_sb = spool.tile([g_, 1], f32)
            nrm_sb = spool.tile([g_, 1], f32)
            inv_sb = spool.tile([g_, 1], f32)
            o_sb = spool.tile([g_, do], f32)
            nc.vector.tensor_mul(sq_sb[:], g_sb[:], g_sb[:])
            nc.vector.tensor_reduce(s_sb[:], sq_sb[:],
                                    axis=mybir.AxisListType.X, op=ALU.add)
            nc.scalar.sqrt(nrm_sb[:], s_sb[:])
            nc.vector.tensor_scalar_max(nrm_sb[:], nrm_sb[:], 1e-12)
            nc.vector.reciprocal(inv_sb[:], nrm_sb[:])
            nc.vector.tensor_scalar_mul(o_sb[:], g_sb[:], inv_sb[:, :1])
            nc.sync.dma_start(out_p[:], o_sb[:])

    nc.compile()
    return nc


_CACHE = {}
_LAST_EXEC_NS = None


def _run(cfg, x, W1, b1, W2, b2, Wl, bl, edge_index, batch, trace=False):
    import ml_dtypes
    from concourse.bass_utils import run_bass_kernel_spmd
    bfd = ml_dtypes.bfloat16

    pre = preprocess(cfg, x, edge_index, batch)
    key = (cfg.n_nodes, cfg.nloc, tuple(pre["ttot"]),
           tuple(tuple(T.tolist()) for T in pre["T"]))
    if key not in _CACHE:
        _CACHE[key] = build(cfg, pre["T"], pre["ttot"])
    nc = _CACHE[key]

    in_maps = []
    for c in range(cfg.n_cores):
        m = {}
        for h in range(cfg.nh):
            m[f"gidx{h}"] = np.ascontiguousarray(pre[f"gidx{h}"][c])
            m[f"dcol{h}"] = np.ascontiguousarray(pre[f"dcol{h}"][c])
        m.update({
            "t1h0": pre["t1h0"],
            "t1h1": pre["t1h1"],
            "pm": np.ascontiguousarray(pre["pm"][c]),
            "dg": np.ascontiguousarray(pre["dg"][c]),
            "dinvc": np.ascontiguousarray(pre["dinvc"][c]),
            "W1": np.asarray(W1, np.float32).astype(bfd),
            "W2": np.asarray(W2, np.float32).astype(bfd),
            "Wl": np.asarray(Wl, np.float32),
            "b1": np.asarray(b1, np.float32).astype(bfd).reshape(1, -1),
            "b2": np.asarray(b2, np.float32).astype(bfd).reshape(1, -1),
            "bl": np.asarray(bl, np.float32).reshape(1, -1),
        })
        in_maps.append(m)
    res = run_bass_kernel_spmd(nc, in_maps, list(range(cfg.n_cores)),
                               trace=trace)
    global _LAST_EXEC_NS
    _LAST_EXEC_NS = res.exec_time_ns
    return np.asarray(res.results[0]["out"], np.float32)


def kernel(x, W1, b1, W2, b2, Wl, bl, edge_index, batch):
    cfg = GCNConfig()
    return _run(cfg, x, W1, b1, W2, b2, Wl, bl, edge_index, batch)


# revision 43
# speedup vs baseline: 1.1224x; 1.1224x over previous
"""Trainium2 Bass kernel for a 2-layer GCN encoder (AssemblyQueryEncoder).

Reference computation (PyG-style GCNConv x2 + global mean pool + linear + L2norm):
    h1 = relu(gcnconv(x, W1, b1));  h2 = relu(gcnconv(h1, W2, b2))
    g  = segment_mean(h2, batch) @ Wl + bl;  out = g / max(||g||_2, eps)

Distribution over 8 NeuronCores:
  - Nodes sharded contiguously (5120 padded/core); each core owns the incoming
    edges of its nodes (destination partitioning).
  - Norm folding: dinv[src] is folded into the gather-table rows (pre-scaled
    x / scaled transpose for layer 2), dinv[dst] is applied as a per-partition
    activation scale on the aggregation PSUM.  Per-edge selection matrices are
    therefore 0/1 one-hot and generated ON-CHIP (DVE is_equal against an iota
    row) from a 2-byte dstcol stream; nothing dense is streamed from DRAM.
  - Self-loops ride as ordinary gathered self-edges.
  - Aggregation is linear, so the weight transform runs AFTER aggregation:
    the layer-1 table is just dinv*x — a pure host-built parameter (zero
    kernel time before gathers start) — and layer 2 gathers raw dinv*h1.
    Each block's epilogue does scaled-transpose (diag-dinv matmul, folding
    dinv[dst]) -> @W -> rank-1 bias matmul -> Relu; psum->bf16 staging
    copies run on the scalar engine so the DVE keeps the one-hot stream.
    Layer 2's table AllGather is split in 2 halves fired mid-layer-1 and its
    aggregation is two-pass (stream 0 vs 1) to hide collective latency.
  - Tables are split in 2 halves (<=20480 rows) so dma_gather int16 indices
    cover them; gathers are issued in 8-tile (1024-index) calls — the SWDGE
    descriptor ring holds exactly 1024 descriptors, larger calls deadlock —
    round-robined over the 4 SWDGE queues with an 8-deep buffer pipeline.
    Pad slots gather spread-out throwaway rows: same-address pad gathers
    serialize the DMA drain and were the dominant cost at one point.
  - Pooled per-graph sums (1/count folded into the pooling matrix) are
    AllReduced ([128,64]); final linear + L2 norm computed redundantly in f32.
"""

import sys

sys.path.insert(0, "/opt/trn_rl_repo")

import numpy as np

P = 128  # partitions


def _cdiv(a, b):
    return (a + b - 1) // b


class GCNConfig:
    def __init__(self, n_nodes=40000, n_graphs=64, d_in=128, d_hid=128, d_out=64,
                 n_cores=8, gch=8, sgen=16):
        self.n_nodes = n_nodes
        self.n_graphs = n_graphs
        self.d_in = d_in
        self.d_hid = d_hid
        self.d_out = d_out
        self.n_cores = n_cores
        self.gch = gch      # gather chunk (tiles per dma_gather call)
        self.sgen = sgen    # one-hot generation chunk (tiles per DVE op)
        self.nloc = _cdiv(n_nodes, n_cores * P) * P  # padded nodes per core
        self.npad = self.nloc * n_cores
        self.nblk = self.nloc // P   # 128-node blocks per core (40)
        self.nh = 2                  # table halves
        self.hs = self.nloc // self.nh           # rows per half per core (2560)
        self.hrows = self.hs * n_cores           # rows per half table (20480)
        assert self.hrows <= 32768  # int16 gather indices
        assert self.hs % P == 0


def _wrap_idx(flat):
    """dma_gather index layout: element i -> [i % 16, i // 16], x8 partitions."""
    n = flat.shape[0]
    assert n % 16 == 0
    arr = np.zeros((16, n // 16), np.int16)
    arr[np.arange(n) % 16, np.arange(n) // 16] = flat
    return np.tile(arr, (8, 1))


def preprocess(cfg, x, edge_index, batch):
    """Host-side preprocessing.  Edges (plus one self-edge per real node) are
    grouped per core by destination block and split into nh streams by source
    half; each (block, stream) list is padded to a tile multiple shared by all
    cores.  Streams carry int16 gather rows + bf16 destination columns."""
    import ml_dtypes
    bfd = ml_dtypes.bfloat16

    n, nc_ = cfg.n_nodes, cfg.n_cores
    nh, hs = cfg.nh, cfg.hs
    src_a = np.asarray(edge_index[0], dtype=np.int64)
    dst_a = np.asarray(edge_index[1], dtype=np.int64)
    batch = np.asarray(batch, dtype=np.int64)

    deg = np.bincount(dst_a, minlength=n).astype(np.float64) + 1.0
    dinv = 1.0 / np.sqrt(deg)

    # append self-edges (i -> i); all folded coefficients are 1 in the
    # one-hot scheme (dinv[src] in the table, dinv[dst] in the epilogue)
    selfs = np.arange(n, dtype=np.int64)
    src_a = np.concatenate([src_a, selfs])
    dst_a = np.concatenate([dst_a, selfs])

    # source half + row within the half table (rank-major concat layout)
    h_a = (src_a % cfg.nloc) // hs
    row_a = (src_a // cfg.nloc) * hs + (src_a % hs)

    order = np.lexsort((dst_a, h_a))
    src_h = h_a[order]
    dst_s = dst_a[order]
    row_s = row_a[order]
    hstart = np.searchsorted(src_h, np.arange(nh + 1))

    nblk_g = cfg.npad // P
    res = {"T": [], "ttot": []}
    for h in range(nh):
        lo_, hi_ = hstart[h], hstart[h + 1]
        s_r = row_s[lo_:hi_]
        s_d = dst_s[lo_:hi_]
        blk = s_d // P
        counts = np.bincount(blk, minlength=nblk_g).reshape(nc_, cfg.nblk)
        T = _cdiv(counts.max(axis=0), P).astype(np.int64)
        ttot = max(int(T.sum()), 1)
        tstart = np.concatenate([[0], np.cumsum(T)]).astype(np.int64)
        bstart = np.concatenate(
            [[0], np.cumsum(np.bincount(blk, minlength=nblk_g))]).astype(np.int64)
        # Pad slots gather a throwaway row (their one-hot column is zeroed);
        # spread them over distinct rows — same-address gathers serialize on
        # one DRAM bank and made the pad-heavy core ~25% slower.
        spread = (np.arange(P)[:, None] * 577 + np.arange(ttot)[None, :] * 131
                  ) % (hs * nc_)
        gidx = np.broadcast_to(spread.astype(np.int16),
                               (nc_, P, ttot)).copy()
        dcol = np.full((nc_, P, ttot), -1.0, bfd)
        for c in range(nc_):
            for b in range(cfg.nblk):
                gb = c * cfg.nblk + b
                e0, e1 = bstart[gb], bstart[gb + 1]
                m = e1 - e0
                if m == 0:
                    continue
                jj = np.arange(m)
                pp, tt = jj % P, tstart[b] + jj // P
                gidx[c, pp, tt] = s_r[e0:e1]
                dcol[c, pp, tt] = (s_d[e0:e1] % P).astype(bfd)
        widx = np.stack([_wrap_idx(gidx[c].T.reshape(-1)) for c in range(nc_)])
        res[f"gidx{h}"] = widx
        res[f"dcol{h}"] = dcol
        res["T"].append(T)
        res["ttot"].append(ttot)

    # per-core constants
    d_all = np.zeros(cfg.npad, np.float64)
    d_all[:n] = dinv
    # dg: per-block diagonal dinv (own nodes) for the scaled transpose
    dg = np.zeros((nc_, P, cfg.nblk * P), bfd)
    # dinvc: [P, nblk] f32 post-aggregation scale (own nodes)
    dinvc = np.zeros((nc_, P, cfg.nblk), np.float32)
    # invd: [1, nloc] bf16 sqrt(deg) for the pre-scaled bias (own nodes)
    invd = np.zeros((nc_, 1, cfg.nloc), bfd)
    for c in range(nc_):
        loc = d_all[c * cfg.nloc:(c + 1) * cfg.nloc]
        for b in range(cfg.nblk):
            dg[c, np.arange(P), b * P + np.arange(P)] = \
                loc[b * P:(b + 1) * P].astype(bfd)
            dinvc[c, :, b] = loc[b * P:(b + 1) * P].astype(np.float32)
        nz = loc > 0
        invd[c, 0, nz] = (1.0 / loc[nz]).astype(bfd)

    # layer-1 gather table: dinv-scaled x rows (aggregation is linear, so
    # the W1 transform moves AFTER aggregation and the table is a pure
    # host-built parameter), rank-major half layout, replicated
    xf = np.asarray(x, dtype=np.float32)
    xsc = np.zeros((cfg.npad, cfg.d_in), bfd)
    xsc[:n] = (xf * dinv[:, None].astype(np.float32)).astype(bfd)
    t1 = [np.zeros((cfg.hrows, cfg.d_in), bfd) for _ in range(nh)]
    for r in range(nc_):
        for h in range(nh):
            t1[h][r * hs:(r + 1) * hs] = \
                xsc[r * cfg.nloc + h * hs:r * cfg.nloc + (h + 1) * hs]

    # pooling matrix with 1/count folded in, block-major [P, nblk*G], bf16
    g_ = cfg.n_graphs
    cnt = np.maximum(np.bincount(batch, minlength=g_).astype(np.float32), 1.0)
    pm = np.zeros((nc_, P, cfg.nblk * g_), bfd)
    for c in range(nc_):
        for b in range(cfg.nblk):
            base = c * cfg.nloc + b * P
            hi2 = min(base + P, n)
            if hi2 <= base:
                continue
            rows = np.arange(hi2 - base)
            gg = batch[base:hi2]
            pm[c, rows, b * g_ + gg] = (1.0 / cnt[gg]).astype(bfd)

    res.update(t1h0=t1[0], t1h1=t1[1], pm=pm, dg=dg, dinvc=dinvc)
    return res


def build(cfg, Ts, ttots):
    """Build the SPMD Bass graph (same program for all cores)."""
    import concourse.mybir as mybir
    import concourse.tile as tile
    from concourse import bacc, bass

    f32 = mybir.dt.float32
    bf = mybir.dt.bfloat16
    i16 = mybir.dt.int16
    AF = mybir.ActivationFunctionType
    ALU = mybir.AluOpType

    nc_ = cfg.n_cores
    nblk = cfg.nblk
    nloc = cfg.nloc
    nh, hs, hrows = cfg.nh, cfg.hs, cfg.hrows
    dh = cfg.d_hid
    do = cfg.d_out
    g_ = cfg.n_graphs
    rg = [list(range(nc_))]
    GCH, SGEN = cfg.gch, cfg.sgen
    tstarts = [np.concatenate([[0], np.cumsum(T)]).astype(np.int64) for T in Ts]

    nc = bacc.Bacc("TRN2", target_bir_lowering=False, debug=False,
                   num_devices=nc_, num_swdge_queues=4)

    # ---- parameters ----
    t1p = [nc.declare_dram_parameter(f"t1h{h}", [hrows, dh], bf,
                                     isOutput=False) for h in range(nh)]
    gidx_p, dcol_p = [], []
    for h in range(nh):
        gidx_p.append(nc.declare_dram_parameter(
            f"gidx{h}", [P, ttots[h] * 8], i16, isOutput=False))
        dcol_p.append(nc.declare_dram_parameter(
            f"dcol{h}", [P, ttots[h]], bf, isOutput=False))
    dg_p = nc.declare_dram_parameter("dg", [P, nblk * P], bf, isOutput=False)
    dinvc_p = nc.declare_dram_parameter("dinvc", [P, nblk], f32, isOutput=False)
    pm_p = nc.declare_dram_parameter("pm", [P, nblk * g_], bf, isOutput=False)
    w1_p = nc.declare_dram_parameter("W1", [cfg.d_in, dh], bf, isOutput=False)
    w2_p = nc.declare_dram_parameter("W2", [dh, dh], bf, isOutput=False)
    wl_p = nc.declare_dram_parameter("Wl", [dh, do], f32, isOutput=False)
    b1_p = nc.declare_dram_parameter("b1", [1, dh], bf, isOutput=False)
    b2_p = nc.declare_dram_parameter("b2", [1, dh], bf, isOutput=False)
    bl_p = nc.declare_dram_parameter("bl", [1, do], f32, isOutput=False)
    out_p = nc.declare_dram_parameter("out", [g_, do], f32, isOutput=True)

    # ---- internal DRAM ----
    ag2h = [nc.dram_tensor(f"ag2h{h}", [hs, dh], bf) for h in range(nh)]
    t2h = [nc.dram_tensor(f"t2h{h}", [hrows, dh], bf, addr_space="Shared")
           for h in range(nh)]
    arin = [nc.dram_tensor(f"arin{i}", [dh, g_], f32) for i in (0, 1)]
    arout = [nc.dram_tensor(f"arout{i}", [dh, g_], f32, addr_space="Shared")
             for i in (0, 1)]

    with tile.TileContext(nc) as tc:
        with (
            tc.tile_pool(name="const", bufs=1) as cpool,
            tc.tile_pool(name="big", bufs=1) as bigpool,
            tc.tile_pool(name="gat", bufs=8) as gpool,
            tc.tile_pool(name="sel", bufs=4) as selpool,
            tc.tile_pool(name="blk", bufs=3) as blkpool,
            tc.tile_pool(name="small", bufs=2) as spool,
            tc.tile_pool(name="psum_a", bufs=3, space="PSUM") as pspool,
            tc.tile_pool(name="psum_e", bufs=2, space="PSUM") as pspool_e,
            tc.tile_pool(name="psum1", bufs=1, space="PSUM") as pspool1,
        ):
            # ---- constants ----
            w1_sb = cpool.tile([cfg.d_in, dh], bf)
            w2_sb = cpool.tile([dh, dh], bf)
            wl_sb = cpool.tile([dh, do], f32)
            b1_sb = cpool.tile([1, dh], bf)
            b2_sb = cpool.tile([1, dh], bf)
            bl_sb = cpool.tile([1, do], f32)
            onesf_sb = cpool.tile([1, P], f32)
            ones_b = cpool.tile([1, P], bf)
            iota_i = cpool.tile([P, P], i16)
            iota_bf = cpool.tile([P, P], bf)
            nc.sync.dma_start(w1_sb[:], w1_p[:])
            nc.sync.dma_start(w2_sb[:], w2_p[:])
            nc.sync.dma_start(wl_sb[:], wl_p[:])
            nc.sync.dma_start(b1_sb[:], b1_p[:])
            nc.sync.dma_start(b2_sb[:], b2_p[:])
            nc.sync.dma_start(bl_sb[:], bl_p[:])
            nc.gpsimd.memset(onesf_sb[:], 1.0)
            nc.gpsimd.memset(ones_b[:], 1.0)
            nc.gpsimd.iota(iota_i[:], pattern=[[1, P]], base=0,
                           channel_multiplier=0)
            nc.gpsimd.tensor_copy(iota_bf[:], iota_i[:])

            gidx_sb, dcol_sb = [], []
            for h in range(nh):
                t = bigpool.tile([P, ttots[h] * 8], i16, tag=f"gidx{h}")
                gidx_sb.append(t)
                t = bigpool.tile([P, ttots[h]], bf, tag=f"dcol{h}")
                dcol_sb.append(t)
            dg_sb = bigpool.tile([P, nblk * P], bf)
            dinvc_sb = bigpool.tile([P, nblk], f32)
            pm_sb = bigpool.tile([P, nblk * g_], bf)
            h2acc = bigpool.tile([P, nloc], f32)

            def bsl(b, w=P):
                return slice(b * w, (b + 1) * w)

            for h in range(nh):
                w4 = (ttots[h] * 8 + 3) // 4
                for q4 in range(4):
                    lo4 = q4 * w4
                    hi4 = min((q4 + 1) * w4, ttots[h] * 8)
                    if hi4 > lo4:
                        nc.scalar.dma_start(gidx_sb[h][:, lo4:hi4],
                                            gidx_p[h][:, lo4:hi4])
                nc.scalar.dma_start(dcol_sb[h][:], dcol_p[h][:])
            nc.scalar.dma_start(dg_sb[:], dg_p[:])
            nc.scalar.dma_start(dinvc_sb[:], dinvc_p[:])
            nc.scalar.dma_start(pm_sb[:], pm_p[:])

            # ---- streamed gather + on-chip one-hot machinery ----
            def make_streams(tables):
                sts = []
                for h in range(nh):
                    sts.append(dict(
                        tstart=tstarts[h], ttot=ttots[h], gidx=gidx_sb[h],
                        dcol=dcol_sb[h], view=tables[h][:],
                        gcur=None, gc0=-1, scur=None, sc0=-1, h=h))
                return sts

            qcnt = [0]

            def fetch(st, t):
                # gather chunk
                c0 = (t // GCH) * GCH
                if st["gc0"] != c0:
                    k = min(GCH, st["ttot"] - c0)
                    gt = gpool.tile([P, GCH, dh], bf, tag="g")
                    nc.gpsimd.dma_gather(
                        out_ap=gt[:, :k, :],
                        in_ap=st["view"],
                        idxs_ap=st["gidx"][:, c0 * 8:(c0 + k) * 8],
                        num_idxs=k * P,
                        num_idxs_reg=k * P,
                        elem_size=dh,
                        queue_num=qcnt[0] % 4,
                    )
                    qcnt[0] += 1
                    st["gcur"], st["gc0"] = gt, c0
                # one-hot selection chunk
                s0 = (t // SGEN) * SGEN
                if st["sc0"] != s0:
                    k2 = min(SGEN, st["ttot"] - s0)
                    sl = selpool.tile([P, SGEN * P], bf, tag="s")
                    dc = st["dcol"][:, s0:s0 + k2]
                    in0 = dc.to_broadcast([P, k2, P])
                    ib = iota_bf[:]
                    in1 = bass.AP(ib.tensor, ib.offset,
                                  [ib.ap[0], [0, k2], ib.ap[1]])
                    out = sl[:, :k2 * P].rearrange("p (a b) -> p a b", b=P)
                    nc.vector.tensor_tensor(out=out, in0=in0, in1=in1,
                                            op=ALU.is_equal)
                    st["scur"], st["sc0"] = sl, s0
                return (st["scur"][:, bsl(t - st["sc0"])],
                        st["gcur"][:, t - st["gc0"], :])

            def pass_a(st0):
                # stream-0-only accumulation of every block into h2acc, so it
                # can run while the other half-table is still being produced
                ts0 = st0["tstart"]
                for b in range(nblk):
                    if ts0[b + 1] == ts0[b]:
                        nc.vector.memset(h2acc[:, bsl(b)], 0.0)
                        continue
                    ps = pspool.tile([P, dh], f32, tag="psa")
                    for t in range(int(ts0[b]), int(ts0[b + 1])):
                        m_ap, g_ap = fetch(st0, t)
                        nc.tensor.matmul(ps[:], m_ap, g_ap,
                                         start=(t == ts0[b]),
                                         stop=(t == ts0[b + 1] - 1))
                    nc.vector.tensor_copy(h2acc[:, bsl(b)], ps[:])

            # ---- layer 1: aggregate raw dinv*x rows, then per-block
            # scaled-transpose (folds dinv[dst]) -> @W1 + bias -> relu; the
            # layer-2 table row dinv*h1 ships via a fused relu-with-scale ----
            streams1 = make_streams(t1p)
            for b in range(nblk):
                ps = pspool.tile([P, dh], f32, tag="psa")
                nt = int(tstarts[0][b + 1] - tstarts[0][b]
                         + tstarts[1][b + 1] - tstarts[1][b])
                k = 0
                for st in streams1:
                    ts = st["tstart"]
                    for t in range(int(ts[b]), int(ts[b + 1])):
                        m_ap, g_ap = fetch(st, t)
                        nc.tensor.matmul(ps[:], m_ap, g_ap,
                                         start=(k == 0), stop=(k == nt - 1))
                        k += 1
                pre_sb = blkpool.tile([P, dh], bf, tag="pre")
                nc.scalar.activation(pre_sb[:], ps[:], AF.Copy)
                psT = pspool_e.tile([P, dh], f32, tag="pse")
                nc.tensor.matmul(psT[:], pre_sb[:], dg_sb[:, bsl(b)],
                                 start=True, stop=True)
                preT = blkpool.tile([P, dh], bf, tag="preT")
                nc.scalar.activation(preT[:], psT[:], AF.Copy)
                ps2 = pspool_e.tile([P, dh], f32, tag="pse")
                nc.tensor.matmul(ps2[:], preT[:], w1_sb[:],
                                 start=True, stop=False)
                nc.tensor.matmul(ps2[:], ones_b[:], b1_sb[:],
                                 start=False, stop=True)
                a2s = blkpool.tile([P, dh], bf, tag="a2s")
                nc.scalar.activation(a2s[:], ps2[:], AF.Relu,
                                     scale=dinvc_sb[:, b:b + 1])
                hh, bb = b // (nblk // nh), b % (nblk // nh)
                nc.sync.dma_start(
                    ag2h[hh][bb * P:(bb + 1) * P, :], a2s[:])
                if b == nblk // nh - 1 or b == nblk - 1:
                    nc.gpsimd.collective_compute(
                        "AllGather", mybir.AluOpType.bypass,
                        replica_groups=rg, ins=[ag2h[hh][:]],
                        outs=[t2h[hh][:]])

            # ---- layer 2 aggregation: two passes ----
            streams2 = make_streams(t2h)
            st0, st1 = streams2
            ts1 = st1["tstart"]
            pass_a(st0)
            psp = pspool1.tile([P, g_], f32, tag="pool")
            for b in range(nblk):  # pass B: stream-1 accumulate + transform
                nt1 = int(ts1[b + 1] - ts1[b])
                if nt1 > 0:
                    ps = pspool.tile([P, dh], f32, tag="psa")
                    for t in range(int(ts1[b]), int(ts1[b + 1])):
                        m_ap, g_ap = fetch(st1, t)
                        nc.tensor.matmul(ps[:], m_ap, g_ap,
                                         start=(t == ts1[b]),
                                         stop=(t == ts1[b + 1] - 1))
                    nc.vector.tensor_tensor(out=h2acc[:, bsl(b)],
                                            in0=h2acc[:, bsl(b)], in1=ps[:],
                                            op=ALU.add)
                pre2 = blkpool.tile([P, dh], bf, tag="pre")
                nc.scalar.activation(pre2[:], h2acc[:, bsl(b)], AF.Copy)
                psT = pspool_e.tile([P, dh], f32, tag="pse")
                nc.tensor.matmul(psT[:], pre2[:], dg_sb[:, bsl(b)],
                                 start=True, stop=True)
                pre2T = blkpool.tile([P, dh], bf, tag="preT")
                nc.scalar.activation(pre2T[:], psT[:], AF.Copy)
                ps3 = pspool_e.tile([P, dh], f32, tag="pse")
                nc.tensor.matmul(ps3[:], pre2T[:], w2_sb[:],
                                 start=True, stop=False)
                nc.tensor.matmul(ps3[:], ones_b[:], b2_sb[:],
                                 start=False, stop=True)
                h2b = blkpool.tile([P, dh], bf, tag="h2")
                nc.scalar.activation(h2b[:], ps3[:], AF.Relu)
                nc.tensor.matmul(psp[:], h2b[:], pm_sb[:, bsl(b, g_)],
                                 start=(b % (nblk // 2) == 0),
                                 stop=(b % (nblk // 2) == nblk // 2 - 1))
                if b == nblk // 2 - 1 or b == nblk - 1:
                    # AllReduce each half of the pooled sums as soon as it is
                    # ready; the first one hides under the remaining blocks
                    half = 0 if b < nblk // 2 else 1
                    pool_sb = spool.tile([dh, g_], f32, tag=f"pl{half}")
                    nc.vector.tensor_copy(pool_sb[:], psp[:])
                    nc.gpsimd.dma_start(arin[half][:], pool_sb[:])
                    nc.gpsimd.collective_compute(
                        "AllReduce", mybir.AluOpType.add, replica_groups=rg,
                        ins=[arin[half][:]], outs=[arout[half][:]])
                    if half == 0:
                        psp = pspool1.tile([P, g_], f32, tag="pool")

            # ---- final linear: partial product per AllReduce half ----
            psg = pspool1.tile([g_, do], f32, tag="fin")
            m0 = spool.tile([dh, g_], f32, tag="m0")
            nc.sync.dma_start(m0[:], arout[0][:])
            nc.tensor.matmul(psg[:], m0[:], wl_sb[:], start=True,
                             stop=False)
            m1 = spool.tile([dh, g_], f32, tag="m1")
            nc.sync.dma_start(m1[:], arout[1][:])
            nc.tensor.matmul(psg[:], m1[:], wl_sb[:], start=False,
                             stop=False)
            nc.tensor.matmul(psg[:], onesf_sb[:, :g_], bl_sb[:],
                             start=False, stop=True)
            g_sb = spool.tile([g_, do], f32)
            nc.vector.tensor_copy(g_sb[:], psg[:])

            # ---- L2 normalize rows ----
            sq_sb = spool.tile([g_, do], f32)
            s_sb = spool.tile([g_, 1], f32)
            nrm_sb = spool.tile([g_, 1], f32)
            inv_sb = spool.tile([g_, 1], f32)
            o_sb = spool.tile([g_, do], f32)
            nc.vector.tensor_mul(sq_sb[:], g_sb[:], g_sb[:])
            nc.vector.tensor_reduce(s_sb[:], sq_sb[:],
                                    axis=mybir.AxisListType.X, op=ALU.add)
            nc.scalar.sqrt(nrm_sb[:], s_sb[:])
            nc.vector.tensor_scalar_max(nrm_sb[:], nrm_sb[:], 1e-12)
            nc.vector.reciprocal(inv_sb[:], nrm_sb[:])
            nc.vector.tensor_scalar_mul(o_sb[:], g_sb[:], inv_sb[:, :1])
            nc.sync.dma_start(out_p[:], o_sb[:])

    nc.compile()
    return nc


_CACHE = {}
_LAST_EXEC_NS = None


def _run(cfg, x, W1, b1, W2, b2, Wl, bl, edge_index, batch, trace=False):
    import ml_dtypes
    from concourse.bass_utils import run_bass_kernel_spmd
    bfd = ml_dtypes.bfloat16

    pre = preprocess(cfg, x, edge_index, batch)
    key = (cfg.n_nodes, cfg.nloc, tuple(pre["ttot"]),
           tuple(tuple(T.tolist()) for T in pre["T"]))
    if key not in _CACHE:
        _CACHE[key] = build(cfg, pre["T"], pre["ttot"])
    nc = _CACHE[key]

    in_maps = []
    for c in range(cfg.n_cores):
        m = {}
        for h in range(cfg.nh):
            m[f"gidx{h}"] = np.ascontiguousarray(pre[f"gidx{h}"][c])
            m[f"dcol{h}"] = np.ascontiguousarray(pre[f"dcol{h}"][c])
        m.update({
            "t1h0": pre["t1h0"],
            "t1h1": pre["t1h1"],
            "pm": np.ascontiguousarray(pre["pm"][c]),
            "dg": np.ascontiguousarray(pre["dg"][c]),
            "dinvc": np.ascontiguousarray(pre["dinvc"][c]),
            "W1": np.asarray(W1, np.float32).astype(bfd),
            "W2": np.asarray(W2, np.float32).astype(bfd),
            "Wl": np.asarray(Wl, np.float32),
            "b1": np.asarray(b1, np.float32).astype(bfd).reshape(1, -1),
            "b2": np.asarray(b2, np.float32).astype(bfd).reshape(1, -1),
            "bl": np.asarray(bl, np.float32).reshape(1, -1),
        })
        in_maps.append(m)
    res = run_bass_kernel_spmd(nc, in_maps, list(range(cfg.n_cores)),
                               trace=trace)
    global _LAST_EXEC_NS
    _LAST_EXEC_NS = res.exec_time_ns
    return np.asarray(res.results[0]["out"], np.float32)


def kernel(x, W1, b1, W2, b2, Wl, bl, edge_index, batch):
    cfg = GCNConfig()
    return _run(cfg, x, W1, b1, W2, b2, Wl, bl, edge_index, batch)
